# revision 25
# baseline (speedup 1.0000x reference)
"""Multi-head attention kernel for Trainium2, 8 NeuronCores (v4).

Problem (hardcoded): B=4, S=2048, E=1024, H=16, DH=64.
  q/k/v = einsum('bse,hed->bhsd', x, W{q,k,v}) + b{q,k,v}
  attn  = softmax(q k^T / sqrt(DH)) v ;  out = concat(attn) @ Wo^T + bo

Sharding: core c -> (batch c//2, head-half c%2: 8 heads, 512 concat cols).
Host sums the two partial out-projections per batch and adds
bo_eff = bo + Wo @ bv_flat (the v-bias commutes through softmax-weighted
averaging, so it is folded into the output bias on the host).

v4 design (cost model: PE charge = out-free rows x cyc/row, fp8 DR = 0.5;
ACT/DVE/Pool charge = free-size x cycle_t; Pool is SBUF-only):
  - projections 3-term fp8 DR (x8.w8 + xr8.w8 + x8.wr8), K in DR chunks
  - q staged as an fp8 pair (q8 + qr8 = bf16-grade): ACT writes qbf from
    PSUM (scale+bias), Pool does the SBUF-side split (copy + subtract);
    k staged single fp8 straight from PSUM (one ACT op) -- k-side
    quantization noise costs ~3e-3 end-to-end (measured in sim)
  - scores: ONE fp8 DR matmul per t-block: qT partitions (q8|qr8),
    kT partitions k8-dup, both slot-broadcast (stride-0) -> psum = 2 q.k
  - exp split ~4.5/8 ACT true-exp + ~3.5/8 DVE Schraudolph
    (i16 = rint(s*SCORE_SCALE*128/ln2 + 16256-7.37) bitcast bf16)
  - normalize_recip on Pool off the ones-column sums; acc/outproj copies
    on DVE; vext scale on ACT; outproj psum in ps_pj (scores keep 3 bufs)
"""

import os
import sys

for _p in ("/opt/trn_rl_repo", "/root/.axon_site/_ro/trn_rl_repo"):
    if os.path.isdir(_p) and _p not in sys.path:
        sys.path.insert(0, _p)
        break

from collections import deque
from contextlib import ExitStack

import numpy as np
import ml_dtypes

import concourse.bass as bass
import concourse.tile as tile
import concourse.mybir as mybir
from concourse import bacc, bass_utils

B, S, E, H, DH = 4, 2048, 1024, 16, 64
HPC = 8             # heads per core
JW = HPC * DH       # 512
SB = S // 128       # 16 s/t-blocks
EB = E // 128       # 8 e-blocks
SC = S // 512       # 4 s-chunks
NJB = JW // 128     # 4 j-blocks
N_CORES = 8

F32 = mybir.dt.float32
BF16 = mybir.dt.bfloat16
FP8 = mybir.dt.float8e4
I16 = mybir.dt.int16
Exp = mybir.ActivationFunctionType.Exp
Ident = mybir.ActivationFunctionType.Identity
ADD = mybir.AluOpType.add
SUB = mybir.AluOpType.subtract
MULT = mybir.AluOpType.mult
DR = mybir.MatmulPerfMode.DoubleRow

NP8 = ml_dtypes.float8_e4m3
NPBF = ml_dtypes.bfloat16

# fp8 e4m3 loses precision near its subnormal range (min normal 2^-6), so
# operands are pre-scaled into mid-range before quantization and the
# descales are folded into existing scalar ops:
XSC = 8.0      # x scaled by 8 on host
WSC = 64.0     # Wq/Wk/Wv scaled by 64 on host
QSC = 16.0     # q/k staged at 16x in fp8
PSUM_TO_QBF = QSC / (XSC * WSC)        # proj psum -> 16x(q+bias), fp8 write
PSUM_TO_V = 64.0 / (XSC * WSC)         # proj psum -> v (x CSC so the
                                       # fp8 concat pair is mid-range)
# scores psum = 2x (q8.k8) from the duplicated DR slots
SCORE_SCALE = 0.125 / (QSC * QSC * 2.0)
CSC = 64.0     # concat staged at 64x in fp8 pair
WOSC = 64.0    # Wo scaled by 64 on host
PO_DESCALE = 1.0 / (CSC * WOSC)

# schraudolph: i16 = rint(arg * 128/ln2 + (127*128 - 7.37))
SCH_C1 = float(SCORE_SCALE * 128.0 / np.log(2.0))
SCH_C2 = float(127.0 * 128.0 - 7.37)
# per-group exp engine: ACT true-exp for ~4.5/8 groups, DVE schraudolph
# for the rest (alternating extra group by iteration parity)
EXP_ACT_EVEN = (True, False, True, False, True, True, False, True)
EXP_ACT_ODD = (True, False, True, True, False, True, False, True)


def _emit(tc, aps, ctx, dbg=None):
    nc = tc.nc
    (x8_d, xr8_d, wq_d, wk_d, wv_d, wo_d, bqc_d, bkc_d, out_d) = aps

    def pool(**kw):
        return ctx.enter_context(tc.tile_pool(**kw))

    const = pool(name="const", bufs=1)
    xp = pool(name="xp", bufs=1)
    q8p = pool(name="q8p", bufs=2)
    qbfp = pool(name="qbf", bufs=2)
    qkT = pool(name="qkT", bufs=6)
    vxp = pool(name="vext", bufs=1)
    exp_p = pool(name="expS", bufs=2)
    anp = pool(name="attn_n", bufs=8)
    asbp = pool(name="acc_sb", bufs=2)
    ccp = pool(name="concatT", bufs=1)
    outp = pool(name="outs", bufs=4)
    ps_sa = pool(name="ps_sa", bufs=2, space="PSUM")   # ACT scores [128,1024]
    ps_sd = pool(name="ps_sd", bufs=1, space="PSUM")   # DVE scores [128,1024]
    ps_ac = pool(name="ps_ac", bufs=1, space="PSUM")   # attn acc [128,512]
    ps_pj = pool(name="ps_pj", bufs=1, space="PSUM")   # proj/outproj [128,512]

    # ---- constants / weights ----
    bqc = const.tile([128, NJB], F32)
    nc.sync.dma_start(bqc[:], bqc_d[:])
    bkc = const.tile([128, NJB], F32)
    nc.sync.dma_start(bkc[:], bkc_d[:])
    wq_sb = const.tile([128, 2, EB, JW], FP8)
    wk_sb = const.tile([128, 2, EB, JW], FP8)
    wv_sb = const.tile([128, 2, EB, JW], FP8)
    wo_sb = const.tile([128, 4, E], BF16)

    def load_wq(ws):
        nc.sync.dma_start(wq_sb[:, ws],
                          wq_d.rearrange("(eb p) w j -> p w eb j", p=128)[:, ws])

    def load_wk(ws):
        nc.sync.dma_start(wk_sb[:, ws],
                          wk_d.rearrange("(eb p) w j -> p w eb j", p=128)[:, ws])

    def load_wv():
        nc.sync.dma_start(wv_sb[:], wv_d.rearrange("(eb p) w j -> p w eb j", p=128))

    def load_wo():
        nc.sync.dma_start(wo_sb[:], wo_d.rearrange("(fb p) e -> p fb e", p=128))

    x8 = xp.tile([128, EB, S], FP8)
    xr8 = xp.tile([128, EB, S], FP8)

    vext = vxp.tile([128, SB, HPC, DH + 1], BF16)

    # 3-term DR chunk list for v: (x operand, w selector)
    TERMS_V = ((x8, 0), (xr8, 0), (x8, 1))

    def emit_v(tb):
        """v for all 8 heads of t-block tb -> vext[:, tb]; the DVE scale op
        is a separate closure so it enqueues after its producer ran."""
        box = {}

        def mms():
            box["pv"] = ps_pj.tile([128, 512], F32, tag="pj", name=f"pv{tb}")
            pv = box["pv"]
            nmm = 3 * (EB // 2)
            k = 0
            for xt, ws in TERMS_V:
                for i in range(EB // 2):
                    nc.tensor.matmul(
                        pv[:], xt[:, 2 * i:2 * i + 2, tb * 128:(tb + 1) * 128],
                        wv_sb[:, ws, 2 * i:2 * i + 2, :],
                        start=(k == 0), stop=(k == nmm - 1), perf_mode=DR)
                    k += 1
            cv_feed.append((tick[0], conv))

        def conv():
            nc.vector.tensor_scalar(
                vext[:, tb, :, 0:DH],
                box["pv"][:].rearrange("p (h d) -> p h d", h=HPC),
                PSUM_TO_V, None, MULT)

        return [mms]

    TERMS = ((x8, 0), (xr8, 0), (x8, 1))
    TERMS_K = ((x8, 0), (x8, 1))

    def emit_qk_chunk(jb, sc, which, tiles, inline_dma=False):
        """One (jb, sc) projection as sub-closures (one DR term each, 4
        matmuls) plus a separate convert closure so the DVE op enqueues
        after its producer ran (avoids head-of-line in the exp stream).
        q: 3 terms + fp8-pair staging; k: 2 terms (x8.(w8+wr8))."""
        qbf, q8, qr8, k8, qT, kT = tiles
        w_sb, bc = ((wq_sb, bqc) if which == "q" else (wk_sb, bkc))
        terms = TERMS if which == "q" else TERMS_K
        nt = len(terms)
        box = {}

        def sub(ti):
            xt, ws = terms[ti]
            if ti == 0:
                box["pq"] = ps_pj.tile([128, 512], F32, tag="pj",
                                       name=f"p{which}{jb}_{sc}")
            pq = box["pq"]
            for i in range(EB // 2):
                nc.tensor.matmul(
                    pq[:], w_sb[:, ws, 2 * i:2 * i + 2, jb * 128:(jb + 1) * 128],
                    xt[:, 2 * i:2 * i + 2, sc * 512:(sc + 1) * 512],
                    start=(ti == 0 and i == 0),
                    stop=(ti == nt - 1 and i == EB // 2 - 1), perf_mode=DR)
            if ti == nt - 1:
                cv_feed.append((tick[0], conv))

        def conv():
            pq = box["pq"]
            sl = slice(sc * 512, (sc + 1) * 512)
            if which == "q":
                nc.vector.tensor_scalar(qbf[:, sl], pq[:], PSUM_TO_QBF,
                                        bc[:, jb:jb + 1], MULT, ADD)
                nc.gpsimd.tensor_copy(q8[:, sl], qbf[:, sl])
                nc.gpsimd.tensor_tensor(qr8[:, sl], qbf[:, sl], q8[:, sl], SUB)
            else:
                nc.vector.tensor_scalar(k8[:, sl], pq[:], PSUM_TO_QBF,
                                        bc[:, jb:jb + 1], MULT, ADD)
            if inline_dma:
                emit_T_dmas(jb, sl, which, tiles)
            elif sc == SC - 1:
                emit_T_dmas(jb, slice(0, S), which, tiles)

        return [lambda ti=ti: sub(ti) for ti in range(nt)]

    def emit_T_dmas(jb, sl, which, tiles):
        """qT: (q8|qr8) partition-stacked; kT: k8 duplicated on halves."""
        qbf, q8, qr8, k8, qT, kT = tiles
        for hl in range(2):
            h, hoff = 2 * jb + hl, hl * 64
            if which == "q":
                nc.sync.dma_start(qT[h][0:64, sl], q8[hoff:hoff + 64, sl])
                nc.sync.dma_start(qT[h][64:128, sl], qr8[hoff:hoff + 64, sl])
            else:
                nc.sync.dma_start(kT[h][0:64, sl], k8[hoff:hoff + 64, sl])
                nc.sync.dma_start(kT[h][64:128, sl], k8[hoff:hoff + 64, sl])

    def emit_jb(jb):
        """Closure list for j-block jb. jb0: per-sc inline DMAs (latency);
        jb>=1: batched full-S DMAs after the last chunk of each side."""
        qbf = qbfp.tile([128, S], BF16, tag="qbf", name=f"qbf{jb}")
        q8 = q8p.tile([128, S], FP8, tag="q8", name=f"q8_{jb}")
        qr8 = q8p.tile([128, S], FP8, tag="qr8", name=f"qr8_{jb}")
        k8 = q8p.tile([128, S], FP8, tag="k8", name=f"k8_{jb}")
        qT, kT = {}, {}
        for hl in range(2):
            h = 2 * jb + hl
            qT[h] = qkT.tile([128, S], FP8, tag="qT", name=f"qT{h}")
            kT[h] = qkT.tile([128, S], FP8, tag="kT", name=f"kT{h}")
            qk_tiles[h] = (qT[h], kT[h])
        tiles = (qbf, q8, qr8, k8, qT, kT)
        inline = jb == 0
        chunks = []
        for sc in range(SC):
            chunks.extend(emit_qk_chunk(jb, sc, "k", tiles, inline))
        for sc in range(SC):
            chunks.extend(emit_qk_chunk(jb, sc, "q", tiles, inline))
        return chunks

    qk_tiles = {}
    concatT = ccp.tile([128, 4, S], BF16)
    attn_n = {}
    pe_feed = deque()
    po_feed = deque()
    cv_feed = deque()          # (stamp, convert closure), min age 2 ticks
    tick = [0]

    def drain(n):
        for _ in range(n):
            if pe_feed:
                pe_feed.popleft()()

    def drain_po(n):
        for _ in range(n):
            if po_feed:
                po_feed.popleft()()

    def emit_outproj(sc, sblk, ec, tail=False):
        def emit():
            off = sc * 512 + sblk * 128
            u = sblk * 2 + ec
            if tail and u >= 2:
                # after the last exp the score banks are free: rotate the
                # remaining tail units across them so they fully pipeline
                sp, tg = ((ps_sa, "sa") if u % 2 == 0 else (ps_sd, "sd"))
                po = sp.tile([128, 1024], F32, tag=tg,
                             name=f"po{sc}_{sblk}_{ec}")[:, 0:512]
            elif tail and u == 1:
                po = ps_ac.tile([128, 512], F32, tag="ac",
                                name=f"po{sc}_{sblk}_{ec}")
            else:
                po = ps_pj.tile([128, 512], F32, tag="pj",
                                name=f"po{sc}_{sblk}_{ec}")
            for fb in range(4):
                nc.tensor.matmul(po[:], concatT[:, fb, off:off + 128],
                                 wo_sb[:, fb, ec * 512:(ec + 1) * 512],
                                 start=(fb == 0), stop=(fb == 3))
            ot = outp.tile([128, 512], F32, tag="ot", name=f"ot{sc}_{sblk}_{ec}")
            nc.vector.tensor_scalar(ot[:], po[:], 1.0 / 64.0, None, MULT)
            nc.sync.dma_start(
                out_d[off:off + 128, ec * 512:(ec + 1) * 512], ot[:])
        return emit

    # ---- attention stream with cross-iteration attnV lag (depth 2: the PE
    # is in-order, so each attnV must trail its exp by enough emitted work
    # that the exp has finished by the time the PE reaches the attnV) ----
    pend = deque()   # (h, sc, tb, expS tile, acc tile)

    def flush_pend():
        if not pend:
            return
        h, sc, tb, eS, acc = pend.popleft()
        # NOTE: a start=True matmul zeroes the whole PSUM bank, so only the
        # very first matmul into this tile may carry it; the other three
        # sblk regions accumulate with start=False onto the zeroed bank.
        for sblk in range(4):
            nc.tensor.matmul(
                acc[:, sblk * 65:sblk * 65 + 65],
                eS[:, tb, sblk * 128:(sblk + 1) * 128],
                vext[:, tb, h, :],
                start=(tb == 0 and sblk == 0), stop=(tb == SB - 1),
                skip_group_check=True)
        if tb == SB - 1:
            finish_iter(h, sc, acc)

    def finish_iter(h, sc, acc):
        acc_sb = asbp.tile([128, 260], F32, tag="asb", name=f"asb{h}_{sc}")
        nc.vector.tensor_copy(acc_sb[:], acc[:, 0:260])
        if h % 2 == 0:
            attn_n[sc] = anp.tile([128, 4, 128], BF16, tag="an",
                                  name=f"an{h}_{sc}")
        an = attn_n[sc]
        hc = (h % 2) * 64
        for sblk in range(4):
            nc.gpsimd.normalize_recip(
                an[:, sblk, hc:hc + 64],
                acc_sb[:, sblk * 65:sblk * 65 + 64],
                acc_sb[:, sblk * 65 + 64:sblk * 65 + 65])
        if h % 2 == 1:
            # SBUF->SBUF XBAR transpose straight into concatT, then the
            # fp8 pair split for the fp8 out-projection (Pool, SBUF-only)
            for sblk in range(4):
                nc.sync.dma_start_transpose(
                    concatT[:, h // 2, sc * 512 + sblk * 128:
                            sc * 512 + (sblk + 1) * 128],
                    an[:, sblk, :])
            if h == HPC - 1:
                for sblk in range(4):
                    for ec in range(2):
                        po_feed.append(emit_outproj(sc, sblk, ec, tail=(sc == SC - 1)))

    def attn_iter(h, sc, it):
        qT, kT = qk_tiles[h]
        eS = exp_p.tile([128, SB, 512], BF16, tag="eS", name=f"eS{h}_{sc}")
        acc = ps_ac.tile([128, 512], F32, tag="ac", name=f"ac{h}_{sc}")
        exp_act = EXP_ACT_EVEN if it % 2 == 0 else EXP_ACT_ODD
        qs = qT[:, sc * 512:(sc + 1) * 512].unsqueeze(1)\
            .broadcast_to((128, 2, 512))
        box = {}
        for tb in range(SB):
            if cv_feed and tick[0] - cv_feed[0][0] >= 2:
                cv_feed.popleft()[1]()
            if tb % 2 == 0:
                sp = ps_sa if exp_act[tb // 2] else ps_sd
                box["scp"] = sp.tile([128, 1024], F32,
                                     tag="sa" if exp_act[tb // 2] else "sd",
                                     name=f"s{h}_{sc}_{tb}")
            scp = box["scp"]
            ks = kT[:, tb * 128:(tb + 1) * 128].unsqueeze(1)\
                .broadcast_to((128, 2, 128))
            nc.tensor.matmul(scp[:, (tb % 2) * 512:(tb % 2 + 1) * 512],
                             ks, qs, start=True, stop=True, perf_mode=DR)
            if tb % 2 == 1:
                dst = eS[:, tb - 1:tb + 1, :].rearrange("p a b -> p (a b)")
                if exp_act[tb // 2]:
                    nc.scalar.activation(dst, scp[:], Exp, scale=SCORE_SCALE)
                else:
                    nc.vector.tensor_scalar(dst.bitcast(I16), scp[:],
                                            SCH_C1, SCH_C2, MULT, ADD)
            if pend and pend[0][0:2] != (h, sc) and tb < 8 and it >= 2:
                flush_pend()
                flush_pend()
            elif len(pend) >= 8:
                flush_pend()
            pend.append((h, sc, tb, eS, acc))
            drain(2 if it < 2 else 1 if tb % 2 == 0 else 0)
            if tb == 11 or tb == 15:
                drain_po(1)
            tick[0] += 1

    # ---- prefix: k j-block 0 first (longest latency chain to the first
    # attention group), early v blocks next, the rest feeds the main loop ----
    nc.gpsimd.memset(vext[:, :, :, DH:DH + 1], 1.0)

    def dma_x8(sc):
        nc.sync.dma_start(
            x8[:, :, sc * 512:(sc + 1) * 512],
            x8_d.rearrange("(eb p) s -> p eb s", p=128)[:, :, sc * 512:(sc + 1) * 512])

    def dma_xr8(sc):
        nc.sync.dma_start(
            xr8[:, :, sc * 512:(sc + 1) * 512],
            xr8_d.rearrange("(eb p) s -> p eb s", p=128)[:, :, sc * 512:(sc + 1) * 512])

    # startup-latency-ordered loads: k0 needs wk+x8(0)+xr8(0); q0 needs
    # wq; k1-3/v need the rest progressively
    load_wk(0)
    dma_x8(0)
    load_wk(1)
    load_wq(0)
    dma_xr8(0)
    load_wq(1)
    for sc in range(1, SC):
        dma_x8(sc)
        dma_xr8(sc)
    load_wv()
    load_wo()

    jb0 = emit_jb(0)
    # sync: k-chunk 0 and q-chunk 0; their converts land in cv_feed --
    # run them inline so iter 0 can start immediately
    for ch in jb0[0:2]:
        ch()
    for ch in jb0[8:11]:
        ch()
    while cv_feed:
        cv_feed.popleft()[1]()
    # feed (matmul subs only; converts self-schedule through cv_feed with
    # a 2-tick lag), cadence 2/tb iters 0-1 then 1 per 2 tb
    vq = [emit_v(tb) for tb in range(SB)]
    pe_feed.extend(jb0[2:6])                     # k1, k2
    pe_feed.extend(vq[0] + vq[1])                # v0, v1
    pe_feed.extend(jb0[6:8])                     # k3
    pe_feed.extend(vq[2] + vq[3] + vq[4])        # v2-v4
    pe_feed.extend(jb0[11:14])                   # q1
    pe_feed.extend(sum(vq[5:16], []))            # v5-v15
    pe_feed.extend(jb0[14:17])                   # q2
    pe_feed.extend(jb0[17:20])                   # q3
    for jb in range(1, NJB):
        pe_feed.extend(emit_jb(jb))

    # ---- main loop ----
    # h-major for h0-3 (j-blocks arrive progressively), then sc-major for
    # h4-7 so each s-chunk's output projection unlocks early and the
    # outproj PE work spreads over iters 20-31 instead of piling up
    iters = [(h, sc) for h in range(4) for sc in range(SC)]
    iters += [(h, sc) for sc in range(SC) for h in range(4, HPC)]
    for it, (h, sc) in enumerate(iters):
        attn_iter(h, sc, it)
    while pend:
        flush_pend()
    drain(len(pe_feed))
    while cv_feed:
        cv_feed.popleft()[1]()
    drain_po(len(po_feed))


_CACHE = {}


def _build():
    nc = bacc.Bacc("TRN2", target_bir_lowering=False, debug=False,
                   num_devices=N_CORES)
    x8_d = nc.dram_tensor("x8", [E, S], FP8, kind="ExternalInput").ap()
    xr8_d = nc.dram_tensor("xr8", [E, S], FP8, kind="ExternalInput").ap()
    wq_d = nc.dram_tensor("wq", [E, 2, JW], FP8, kind="ExternalInput").ap()
    wk_d = nc.dram_tensor("wk", [E, 2, JW], FP8, kind="ExternalInput").ap()
    wv_d = nc.dram_tensor("wv", [E, 2, JW], FP8, kind="ExternalInput").ap()
    wo_d = nc.dram_tensor("wo", [JW, E], BF16, kind="ExternalInput").ap()
    bqc_d = nc.dram_tensor("bqc", [128, NJB], F32, kind="ExternalInput").ap()
    bkc_d = nc.dram_tensor("bkc", [128, NJB], F32, kind="ExternalInput").ap()
    out_d = nc.dram_tensor("out", [S, E], F32, kind="ExternalOutput").ap()
    aps = (x8_d, xr8_d, wq_d, wk_d, wv_d, wo_d, bqc_d, bkc_d, out_d)
    with tile.TileContext(nc) as tc:
        with ExitStack() as ctx:
            _emit(tc, aps, ctx)
    nc.compile()
    return nc


def _prep_core(x, Wq, bq, Wk, bk, Wv, bv, Wo, c):
    b, hh = c // 2, c % 2
    hs = slice(hh * HPC, (hh + 1) * HPC)
    xT = np.ascontiguousarray(x[b].T) * np.float32(XSC)     # [E, S]
    x8 = xT.astype(NP8)
    xr8 = (xT - x8.astype(np.float32)).astype(NP8)

    def wpair(W):
        Wc = np.ascontiguousarray(
            W[hs].transpose(1, 0, 2).reshape(E, JW)) * np.float32(WSC)
        w8 = Wc.astype(NP8)
        wr8 = (Wc - w8.astype(np.float32)).astype(NP8)
        return np.ascontiguousarray(np.stack([w8, wr8], axis=1))  # [E, 2, JW]

    WoT = np.ascontiguousarray(Wo.T)                        # [f, e]

    def wopair(W):
        Wc = np.ascontiguousarray(W) * np.float32(WOSC)
        w8 = Wc.astype(NP8)
        wr8 = (Wc - w8.astype(np.float32)).astype(NP8)
        return np.ascontiguousarray(np.stack([w8, wr8], axis=1))  # [f, 2, e]
    return {
        "x8": x8, "xr8": xr8,
        "wq": wpair(Wq), "wk": wpair(Wk), "wv": wpair(Wv),
        "wo": np.ascontiguousarray(
            WoT[hh * JW:(hh + 1) * JW]).astype(NPBF),
        "bqc": np.ascontiguousarray(
            bq[hs].reshape(NJB, 128).T * np.float32(QSC)),
        "bkc": np.ascontiguousarray(
            bk[hs].reshape(NJB, 128).T * np.float32(QSC)),
    }


def kernel(x, Wq, bq, Wk, bk, Wv, bv, Wo, bo):
    x = np.asarray(x, dtype=np.float32)
    Wq = np.asarray(Wq, dtype=np.float32)
    bq = np.asarray(bq, dtype=np.float32)
    Wk = np.asarray(Wk, dtype=np.float32)
    bk = np.asarray(bk, dtype=np.float32)
    Wv = np.asarray(Wv, dtype=np.float32)
    bv = np.asarray(bv, dtype=np.float32)
    Wo = np.asarray(Wo, dtype=np.float32)
    bo = np.asarray(bo, dtype=np.float32)

    if "nc" not in _CACHE:
        _CACHE["nc"] = _build()
    nc = _CACHE["nc"]

    in_maps = [_prep_core(x, Wq, bq, Wk, bk, Wv, bv, Wo, c)
               for c in range(N_CORES)]
    res = bass_utils.run_bass_kernel_spmd(nc, in_maps,
                                          core_ids=list(range(N_CORES)))
    bo_eff = bo + Wo @ bv.reshape(-1)
    out = np.empty((B, S, E), dtype=np.float32)
    for b in range(B):
        out[b] = res.results[2 * b]["out"] + res.results[2 * b + 1]["out"]
        out[b] += bo_eff[None, :]
    return out
